# revision 19
# baseline (speedup 1.0000x reference)
"""ArcFace loss on 8 TRN2 NeuronCores, tensor-parallel over the class dim.

Reference computation (B=1024, D=512, C=100000):
    e = l2norm(embeddings); w = l2norm(weight)
    cos = clip(e @ w.T);  phi = cos(theta + m) with easy-margin fallback
    logits = S * (onehot*phi + (1-onehot)*cos);  loss = mean CE

Distribution: classes sharded 12500/core. Each core computes its partial
sum-of-exp Z_b over its class shard with a fused matmul->exp(scale_b * x)
pipeline (scale_b = S/||e_b|| folded into the Scalar-engine activation,
1/||w_c|| folded into the bf16 weight tiles before the matmul). The
softmax shift is not needed: |cos|<=1 so S*cos in [-64, 64] and
exp() stays in fp32 range. Partial Z vectors (4KB) are AllGathered and
summed; the target-logit path (cos to the label's weight row) is computed
redundantly on every core in fp32 from host-gathered weight rows.

    Z[b]   = sum_c exp(S*cos[b,c])          (allreduced over cores)
    nll[b] = log(Z - exp(S*cos_t) + exp(S*phi)) - S*phi
    loss   = mean_b nll[b]
"""

import math

import numpy as np
import ml_dtypes

import concourse.bass as bass
import concourse.bass_isa as bass_isa
import concourse.tile as tile
from concourse import bacc, mybir
from concourse.bass_utils import run_bass_kernel_spmd
from concourse.masks import make_identity

# problem shapes (hardcoded per spec)
B, D, C = 1024, 512, 100000
N_CORES = 8
CS = C // N_CORES            # 12500 classes per core
NBT = B // 128               # 8 batch tiles
NKT = D // 128               # 4 contraction tiles
CHUNK = 512                  # matmul free-dim chunk (4 class-tiles of 128)
N_CHUNKS = (CS + CHUNK - 1) // CHUNK   # 25 (last chunk 212 wide)

# arcface constants
S = 64.0
M = 0.5
COS_M = math.cos(M)
SIN_M = math.sin(M)
TH = math.cos(math.pi - M)
MM_ = math.sin(math.pi - M) * M
EPS = 1e-7

F32 = mybir.dt.float32
BF16 = mybir.dt.bfloat16

_NC_CACHE = []


def _build(finalize=True):
    nc = bacc.Bacc(num_devices=N_CORES)

    wt = nc.declare_dram_parameter("wt", [D, CS], BF16, isOutput=False)
    wr = nc.declare_dram_parameter("wr", [CS, D], BF16, isOutput=False)
    et = nc.declare_dram_parameter("et", [D, B], BF16, isOutput=False)
    er = nc.declare_dram_parameter("er", [B, D], F32, isOutput=False)
    wlab = nc.declare_dram_parameter("wlab", [B, D], F32, isOutput=False)
    out_ext = nc.declare_dram_parameter("out", [1, 1], F32, isOutput=True)

    zc_in = nc.dram_tensor("zc_in", [128, NBT], F32)
    zc_out = nc.dram_tensor("zc_out", [128 * N_CORES, NBT], F32, addr_space="Shared")

    with tile.TileContext(nc) as tc:
        with (
            tc.tile_pool(name="singles", bufs=1) as singles,
            tc.tile_pool(name="rowp", bufs=2) as rowp,
            tc.tile_pool(name="psumT", bufs=2, space="PSUM") as psumTp,
            tc.tile_pool(name="work", bufs=3) as work,
            tc.tile_pool(name="wrp", bufs=8) as wrp,
            tc.tile_pool(name="sqp", bufs=2) as sqp,
            tc.tile_pool(name="ncolp", bufs=4) as ncolp,
            tc.tile_pool(name="bcp", bufs=4) as bcp,
            tc.tile_pool(name="wtcp", bufs=4) as wtcp,
            tc.tile_pool(name="wtsp", bufs=4) as wtsp,
            tc.tile_pool(name="junkp", bufs=2) as junkp,
            tc.tile_pool(name="tiny", bufs=1) as tiny,
            tc.tile_pool(name="psum", bufs=6, space="PSUM") as psump,
            tc.tile_pool(name="dramp", bufs=3, space="DRAM") as dramp,
        ):
            # ---- constants / replicated inputs ----
            ident = singles.tile([128, 128], F32)
            make_identity(nc, ident)

            et_s = singles.tile([128, NKT, B], BF16)
            for k in range(NKT):
                nc.sync.dma_start(out=et_s[:, k, :], in_=et[k * 128:(k + 1) * 128, :])

            # ---- embedding norms + target-logit path (fp32, replicated) ----
            ne2 = tiny.tile([128, NBT], F32)   # ||e||^2
            nl2 = tiny.tile([128, NBT], F32)   # ||w_label||^2
            dt_ = tiny.tile([128, NBT], F32)   # e . w_label
            for b in range(NBT):
                er_t = work.tile([128, D], F32)
                nc.sync.dma_start(out=er_t, in_=er[b * 128:(b + 1) * 128, :])
                wl_t = work.tile([128, D], F32)
                nc.sync.dma_start(out=wl_t, in_=wlab[b * 128:(b + 1) * 128, :])
                sq = work.tile([128, D], F32)
                nc.vector.tensor_mul(sq, er_t, er_t)
                nc.vector.tensor_reduce(out=ne2[:, b:b + 1], in_=sq,
                                        axis=mybir.AxisListType.X,
                                        op=mybir.AluOpType.add)
                sq2 = work.tile([128, D], F32)
                nc.vector.tensor_mul(sq2, wl_t, wl_t)
                nc.vector.tensor_reduce(out=nl2[:, b:b + 1], in_=sq2,
                                        axis=mybir.AxisListType.X,
                                        op=mybir.AluOpType.add)
                sq3 = work.tile([128, D], F32)
                nc.vector.tensor_mul(sq3, er_t, wl_t)
                nc.vector.tensor_reduce(out=dt_[:, b:b + 1], in_=sq3,
                                        axis=mybir.AxisListType.X,
                                        op=mybir.AluOpType.add)

            nrm_e = tiny.tile([128, NBT], F32)
            nc.scalar.activation(out=nrm_e, in_=ne2,
                                 func=mybir.ActivationFunctionType.Sqrt)
            inv_e = tiny.tile([128, NBT], F32)
            nc.vector.reciprocal(out=inv_e, in_=nrm_e)
            se = tiny.tile([128, NBT], F32)      # S / ||e_b|| : exp scale
            nc.vector.tensor_scalar_mul(se, inv_e, S)

            nrm_l = tiny.tile([128, NBT], F32)
            nc.scalar.activation(out=nrm_l, in_=nl2,
                                 func=mybir.ActivationFunctionType.Sqrt)
            inv_l = tiny.tile([128, NBT], F32)
            nc.vector.reciprocal(out=inv_l, in_=nrm_l)

            cost = tiny.tile([128, NBT], F32)    # cos(theta) to target
            nc.vector.tensor_mul(cost, dt_, inv_e)
            nc.vector.tensor_mul(cost, cost, inv_l)
            nc.vector.tensor_scalar_min(cost, cost, 1.0 - EPS)
            nc.vector.tensor_scalar_max(cost, cost, -1.0 + EPS)

            c2 = tiny.tile([128, NBT], F32)
            nc.vector.tensor_mul(c2, cost, cost)
            sint = tiny.tile([128, NBT], F32)    # sqrt(1 - cos^2)
            nc.scalar.activation(out=sint, in_=c2,
                                 func=mybir.ActivationFunctionType.Sqrt,
                                 bias=1.0, scale=-1.0)
            pa = tiny.tile([128, NBT], F32)
            nc.vector.tensor_scalar_mul(pa, cost, COS_M)
            pb = tiny.tile([128, NBT], F32)
            nc.vector.tensor_scalar_mul(pb, sint, SIN_M)
            phi = tiny.tile([128, NBT], F32)
            nc.vector.tensor_sub(phi, pa, pb)
            msk = tiny.tile([128, NBT], F32)
            nc.vector.tensor_scalar(out=msk, in0=cost, scalar1=TH, scalar2=None,
                                    op0=mybir.AluOpType.is_gt)
            alt = tiny.tile([128, NBT], F32)
            nc.vector.tensor_scalar_sub(alt, cost, MM_)
            dd = tiny.tile([128, NBT], F32)
            nc.vector.tensor_sub(dd, phi, alt)
            md = tiny.tile([128, NBT], F32)
            nc.vector.tensor_mul(md, msk, dd)
            phif = tiny.tile([128, NBT], F32)   # where(cos>TH, phi, cos-MM)
            nc.vector.tensor_add(phif, alt, md)
            st = tiny.tile([128, NBT], F32)      # S * phi  (target logit)
            nc.vector.tensor_scalar_mul(st, phif, S)
            ect = tiny.tile([128, NBT], F32)     # exp(S * cos_t)
            nc.scalar.activation(out=ect, in_=cost,
                                 func=mybir.ActivationFunctionType.Exp, scale=S)
            ept = tiny.tile([128, NBT], F32)     # exp(S * phi)
            nc.scalar.activation(out=ept, in_=st,
                                 func=mybir.ActivationFunctionType.Exp)

            # ---- main pipeline over class chunks ----
            # Norms are produced in groups of NG chunks: fused square+reduce
            # on row-major wr tiles -> [128, 4*NG] columns of ||w||^2, one PE
            # transpose to rows, bounce to DRAM, so each chunk's broadcast
            # read-back is a flat contiguous 512-float line replicated over
            # partitions (the only descriptor-efficient broadcast pattern).
            zparts = singles.tile([128, N_CHUNKS, NBT], F32)
            NG = 5
            for g in range((N_CHUNKS + NG - 1) // NG):
                g0 = g * NG
                gchunks = min(NG, N_CHUNKS - g0)
                gt0 = g0 * 4                        # first class-tile of group
                gnt = min(4 * NG, (CS - g0 * CHUNK + 127) // 128)  # tiles in grp

                ncol = ncolp.tile([128, 4 * NG], F32)
                sqw = sqp.tile([128, 4 * NG, D], BF16)
                if gnt < 4 * NG or (CS - (gt0 + gnt - 1) * 128) < 128:
                    # ragged tail: init unwritten lanes so sqrt stays finite
                    nc.vector.memset(sqw[:, gnt - 1:, :], 1.0 / D)
                for j in range(gnt):
                    t0 = (gt0 + j) * 128
                    w = min(128, CS - t0)
                    wr_t = wrp.tile([128, D], BF16)
                    nc.sync.dma_start(out=wr_t[:w], in_=wr[t0:t0 + w, :])
                    nc.gpsimd.tensor_tensor(out=sqw[:w, j, :], in0=wr_t[:w],
                                            in1=wr_t[:w],
                                            op=mybir.AluOpType.mult)
                nc.vector.tensor_reduce(out=ncol, in_=sqw,
                                        axis=mybir.AxisListType.X,
                                        op=mybir.AluOpType.add)
                nrm_c = ncolp.tile([128, 4 * NG], F32)
                nc.scalar.activation(out=nrm_c, in_=ncol,
                                     func=mybir.ActivationFunctionType.Sqrt)
                invc = ncolp.tile([128, 4 * NG], F32)
                nc.vector.reciprocal(out=invc, in_=nrm_c)
                psT = psumTp.tile([4 * NG, 128], F32)
                nc.tensor.transpose(out=psT, in_=invc, identity=ident)
                rowS = rowp.tile([4 * NG, 128], F32)
                nc.scalar.copy(out=rowS, in_=psT)
                dinv = dramp.tile([4 * NG, 128], F32)
                nc.sync.dma_start(out=dinv[:, :], in_=rowS)
                dap = dinv[:, :]

                for ctl in range(gchunks):
                    ct = g0 + ctl
                    c0 = ct * CHUNK
                    cw = min(CHUNK, CS - c0)           # 512 or 212

                    # bc[p, c'] = 1/||w_{c0+c'}||, all partitions
                    bc = bcp.tile([128, CHUNK], F32)
                    nc.sync.dma_start(
                        out=bc,
                        in_=bass.AP(tensor=dap.tensor,
                                    offset=dap.offset + ctl * CHUNK,
                                    ap=[[0, 128], [1, CHUNK]]))

                    # weight tiles, scaled by 1/||w_c|| (bf16)
                    wtc = wtcp.tile([128, NKT, CHUNK], BF16)
                    for k in range(NKT):
                        nc.sync.dma_start(out=wtc[:, k, :cw],
                                          in_=wt[k * 128:(k + 1) * 128, c0:c0 + cw])
                    wts = wtsp.tile([128, NKT, CHUNK], BF16)
                    for k in range(NKT):
                        nc.gpsimd.tensor_tensor(
                            out=wts[:, k, :cw], in0=wtc[:, k, :cw],
                            in1=bc[:, :cw], op=mybir.AluOpType.mult)

                    # matmul + exp per batch tile; one batched z-reduce
                    junk = junkp.tile([128, NBT, CHUNK], BF16)
                    for b in range(NBT):
                        ps = psump.tile([128, CHUNK], F32)
                        for k in range(NKT):
                            nc.tensor.matmul(
                                out=ps[:, :cw],
                                lhsT=et_s[:, k, b * 128:(b + 1) * 128],
                                rhs=wts[:, k, :cw],
                                start=(k == 0), stop=(k == NKT - 1))
                        nc.scalar.activation(
                            out=junk[:, b, :cw], in_=ps[:, :cw],
                            func=mybir.ActivationFunctionType.Exp,
                            scale=se[:, b:b + 1])
                    nc.vector.tensor_reduce(
                        out=zparts[:, ct, :], in_=junk[:, :, :cw],
                        axis=mybir.AxisListType.X, op=mybir.AluOpType.add)

            # ---- combine partial Z, allgather, final loss ----
            zloc = tiny.tile([128, NBT], F32)
            nc.vector.tensor_reduce(
                out=zloc, in_=zparts.rearrange("p c b -> p b c"),
                axis=mybir.AxisListType.X, op=mybir.AluOpType.add)
            nc.sync.dma_start(out=zc_in[:, :], in_=zloc)
            nc.gpsimd.collective_compute(
                "AllGather", mybir.AluOpType.bypass,
                replica_groups=[list(range(N_CORES))],
                ins=[zc_in[:, :]], outs=[zc_out[:, :]])
            zg = tiny.tile([128, NBT, N_CORES], F32)
            for g in range(N_CORES):
                nc.sync.dma_start(out=zg[:, :, g],
                                  in_=zc_out[g * 128:(g + 1) * 128, :])
            zfull = tiny.tile([128, NBT], F32)
            nc.vector.tensor_reduce(out=zfull, in_=zg,
                                    axis=mybir.AxisListType.X,
                                    op=mybir.AluOpType.add)
            # Zmod = Z - exp(S cos_t) + exp(S phi);  nll = ln(Zmod) - S phi
            nc.vector.tensor_sub(zfull, zfull, ect)
            nc.vector.tensor_add(zfull, zfull, ept)
            lg = tiny.tile([128, NBT], F32)
            nc.scalar.activation(out=lg, in_=zfull,
                                 func=mybir.ActivationFunctionType.Ln)
            nll = tiny.tile([128, NBT], F32)
            nc.vector.tensor_sub(nll, lg, st)
            nll1 = tiny.tile([128, 1], F32)
            nc.vector.tensor_reduce(out=nll1, in_=nll,
                                    axis=mybir.AxisListType.X,
                                    op=mybir.AluOpType.add)
            nllr = tiny.tile([128, 1], F32)
            nc.gpsimd.partition_all_reduce(nllr[:, :], nll1[:, :], 128, bass_isa.ReduceOp.add)
            res = tiny.tile([1, 1], F32)
            nc.scalar.mul(out=res, in_=nllr[0:1, 0:1], mul=1.0 / B)
            nc.sync.dma_start(out=out_ext[:, :], in_=res)

    if finalize:
        nc.finalize()
    return nc


def _get_nc():
    if not _NC_CACHE:
        _NC_CACHE.append(_build())
    return _NC_CACHE[0]


def kernel(embeddings, labels, weight):
    e = np.ascontiguousarray(np.asarray(embeddings, dtype=np.float32))
    w = np.ascontiguousarray(np.asarray(weight, dtype=np.float32))
    lab = np.asarray(labels).astype(np.int64)

    wlab_np = np.ascontiguousarray(w[lab])                       # [B, D] f32
    et_np = np.ascontiguousarray(e.T).astype(ml_dtypes.bfloat16)  # [D, B]
    wt_full = w.T.astype(ml_dtypes.bfloat16)                     # [D, C]

    in_maps = []
    for i in range(N_CORES):
        sl = slice(CS * i, CS * (i + 1))
        in_maps.append({
            "wt": np.ascontiguousarray(wt_full[:, sl]),
            "wr": np.ascontiguousarray(w[sl]).astype(ml_dtypes.bfloat16),
            "et": et_np,
            "er": e,
            "wlab": wlab_np,
        })

    nc = _get_nc()
    res = run_bass_kernel_spmd(nc, in_maps, list(range(N_CORES)))
    out = np.asarray(res.results[0]["out"], dtype=np.float32).reshape(())
    return out
